# revision 27
# baseline (speedup 1.0000x reference)
"""Trainium2 Bass kernel for ConvPixelToCapsules (conv -> 3-iter dynamic routing).

Strategy (hardcoded for x[8,32,8,32,32], conv_w[256,8,3,3], bias[32,8,1,1]):
  - Host precomputes im2col patches per batch element, with an extra 33rd
    "channel" slot holding sum_ci(x) (conv linearity gives iteration-1's
    uniform-route preactivation for free), plus the weight matrix in
    [72, (no,co)] layout and a partition-broadcast bias tile.
  - 8 NeuronCores, data-parallel over batch: core k owns batch element k.
  - Per core: 8 tiles of 128 output pixels. Per tile: 33 matmuls
    (stationary = patches[72,128], moving = w[72,256]) put votes directly in
    [pixel-partition; (ci,no,co)] layout in PSUM -> SBUF. Routing math is
    free-dim ops (softmax over co, reduce over ci, squash over no,
    distances over no); votes never leave SBUF. Final activations are
    PE-transposed so the HBM write is fully contiguous.
  - v4: every segmented reduction (sum over ci for preactivations, sum over
    no for logit distances) runs on the PE as a chain of PSUM-accumulated
    identity matmuls (lhsT = identity, moving = the bf16 product slices,
    start/stop bracketing the chain) -> fp32 accumulation on the otherwise
    idle tensor engine. The DVE keeps only the four elementwise product
    passes (bf16 2x mode) + softmax/squash small ops. Products and their
    PE accumulations are interleaved per ci-half so DVE and PE pipeline
    within one tile; several tiles are kept in flight so the big DVE
    products of one tile fill the small-op phases of the others.
    Logits stay resident in PSUM across routing iterations: iteration 2's
    distance matmuls continue the same accumulation group, so L2 = L1 + D2
    never materializes in SBUF. Squash and softmax read their inputs
    (S, L) straight from PSUM.
  - PSUM evacuations ride the Activation engine; squash's x^2 uses ACT
    Square; sqrt is exp(0.5*ln(x)) so one activation-table set
    (natural_log_exp_and_others) covers every ACT function -> a single
    LoadActFuncSet for the whole program (enforced by narrowing the
    table map handed to the table-load placement pass).
"""

import numpy as np

BS, CI, NI, H, W = 8, 32, 8, 32, 32
CO, NO = 32, 8
NPIX = H * W            # 1024
TILES = 8               # tiles of 128 pixels per batch element
TP = 128                # pixels per tile (on partitions)
K = 72                  # ni * 3 * 3 contraction
SLOTS = CI + 1          # 32 ci + xsum slot
OUTCH = NO * CO         # 256, (no, co) order

CFG = {
    "depth": 3,            # tiles in flight (software pipeline depth)
    "stagger": 8,
    "conv_lag": 26,         # emission rounds between routing admissions
    "pconv_bufs": 2,
    "vmul_pool": False,    # squash V-mul on Pool engine (latency-critical)
    "skip_zero_bias": True,  # compiled-out bias adds when bias is all-zero
    "votes_bufs": 5,
    "big_bufs": 3,
    "state_bufs": 3,
    "pat_bufs": 3,
    "plog_bufs": 2,
    "pacc_bufs": 1,
    "fuse_halves": True,
}

_BUILT = {}


def _host_prep(x, conv_w, bias):
    import ml_dtypes
    x = np.asarray(x, np.float32)
    conv_w = np.asarray(conv_w, np.float32)
    bias = np.asarray(bias, np.float32)
    x_pad = np.pad(x, ((0, 0), (0, 0), (0, 0), (1, 1), (1, 1)))
    x_aug = np.concatenate([x_pad, x_pad.sum(1, keepdims=True)], axis=1)
    wv = np.lib.stride_tricks.sliding_window_view(x_aug, (3, 3), axis=(3, 4))
    cdt_np = ml_dtypes.bfloat16
    patches = np.ascontiguousarray(
        wv.transpose(0, 2, 5, 6, 1, 3, 4).reshape(BS, K, SLOTS, NPIX)
    ).astype(cdt_np)
    w_m = np.ascontiguousarray(
        conv_w.reshape(CO, NO, NI, 3, 3).transpose(2, 3, 4, 1, 0).reshape(K, OUTCH)
    ).astype(cdt_np)
    bias_bc = np.broadcast_to(
        bias[:, :, 0, 0].T.reshape(1, OUTCH), (128, OUTCH)
    ).astype(np.float32)
    ident = np.eye(128, dtype=np.float32)
    ident_bf = np.eye(128, dtype=cdt_np)
    return patches, w_m, bias_bc, ident, ident_bf


def _patch_act_tables():
    """Narrow the activation-table map so every ACT function resolves to
    natural_log_exp_and_others (which holds exp/ln/copy/identity/square):
    the placement pass then emits exactly one LoadActFuncSet, with the
    set id still matching the real act_info.json ordering."""
    import concourse.bacc as bacc_mod
    import concourse.hw_specs as hw_specs
    if getattr(bacc_mod, "_act_tables_narrowed", False):
        return
    real = hw_specs.get_activation_tables
    import functools

    @functools.cache
    def narrowed(arch):
        tables = dict(real(arch))
        keep = "natural_log_exp_and_others"
        assert keep in tables
        return {
            name: (funcs if name == keep else type(funcs)())
            for name, funcs in tables.items()
        }

    bacc_mod.get_activation_tables = narrowed
    bacc_mod._act_tables_narrowed = True


def _build_nc(zero_bias=True):
    key = ("nc", zero_bias) + tuple(
        sorted((k, str(v)) for k, v in CFG.items()))
    if key in _BUILT:
        return _BUILT[key]
    import concourse.bacc as bacc
    import concourse.tile as tile
    import concourse.mybir as mybir

    _patch_act_tables()

    f32 = mybir.dt.float32
    bf16 = mybir.dt.bfloat16
    AF = mybir.ActivationFunctionType
    OP = mybir.AluOpType
    AX = mybir.AxisListType

    nc = bacc.Bacc("TRN2", target_bir_lowering=False, debug=False, num_devices=8)

    use_bias = not (zero_bias and CFG["skip_zero_bias"])

    patches_d = nc.dram_tensor("patches", [K, SLOTS, NPIX], bf16, kind="ExternalInput")
    w_d = nc.dram_tensor("w", [K, OUTCH], bf16, kind="ExternalInput")
    bias_d = nc.dram_tensor("bias", [128, OUTCH], f32, kind="ExternalInput")
    ident_d = nc.dram_tensor("ident", [128, 128], f32, kind="ExternalInput")
    identb_d = nc.dram_tensor("identb", [128, 128], bf16, kind="ExternalInput")
    out_d = nc.dram_tensor("out", [2, 128, NPIX], f32, kind="ExternalOutput")

    with tile.TileContext(nc) as tc:
        with (
            tc.tile_pool(name="const", bufs=1) as const,
            tc.tile_pool(name="pat", bufs=CFG["pat_bufs"]) as patp,
            tc.tile_pool(name="votes", bufs=CFG["votes_bufs"]) as votesp,
            tc.tile_pool(name="big", bufs=CFG["big_bufs"]) as bigp,
            tc.tile_pool(name="state", bufs=CFG["state_bufs"]) as statep,
            tc.tile_pool(name="obuf", bufs=1) as obufp,
            tc.tile_pool(name="pconv", bufs=CFG["pconv_bufs"], space="PSUM") as pconv,
            tc.tile_pool(name="ptr", bufs=1, space="PSUM") as ptr,
            tc.tile_pool(name="pacc", bufs=CFG["pacc_bufs"], space="PSUM") as pacc,
            tc.tile_pool(name="plog", bufs=CFG["plog_bufs"], space="PSUM") as plog,
        ):
            w_sb = const.tile([K, OUTCH], bf16)
            nc.sync.dma_start(w_sb[:], w_d.ap())
            ident_sb = const.tile([128, 128], f32)
            nc.sync.dma_start(ident_sb[:], ident_d.ap())
            identb_sb = const.tile([128, 128], bf16)
            nc.sync.dma_start(identb_sb[:], identb_d.ap())
            if use_bias:
                bias_sb = const.tile([128, OUTCH], f32)
                nc.sync.dma_start(bias_sb[:], bias_d.ap())
            eps_sb = const.tile([128, 1], f32)
            nc.gpsimd.memset(eps_sb[:], 1e-30)

            ob = [
                obufp.tile([128, NPIX], f32, tag=f"ob{h}", name=f"ob{h}")
                for h in range(2)
            ]

            def conv_tile(t):
                # votes for 128 pixels; Uxs slot first so iteration 1 can
                # start before the full evacuation.
                pt = patp.tile([K, SLOTS, TP], bf16, tag="pt", name=f"pt{t}")
                nc.sync.dma_start(
                    pt[:, CI, :], patches_d.ap()[:, CI, t * TP : (t + 1) * TP]
                )
                nc.sync.dma_start(
                    pt[:, :CI, :], patches_d.ap()[:, :CI, t * TP : (t + 1) * TP]
                )
                U = votesp.tile([128, CI, NO, CO], bf16, tag="U", name=f"U{t}")
                Uxs = votesp.tile([128, OUTCH], f32, tag="Uxs", name=f"Uxs{t}")
                conv_tile.out[t] = (U, Uxs)
                pvx = pconv.tile([128, 2, OUTCH], f32, tag="pv",
                                 name=f"pvx{t}")
                nc.tensor.matmul(
                    pvx[:, 0], pt[:, CI, :], w_sb[:], start=True, stop=True)
                nc.scalar.copy(Uxs[:], pvx[:, 0])
                yield
                for k in range(CI // 2):
                    pv = pconv.tile([128, 2, OUTCH], f32, tag="pv",
                                    name=f"pv{t}_{k}")
                    nc.tensor.matmul(
                        pv[:, 0], pt[:, 2 * k, :], w_sb[:],
                        start=True, stop=True)
                    nc.tensor.matmul(
                        pv[:, 1], pt[:, 2 * k + 1, :], w_sb[:],
                        start=True, stop=True)
                    nc.scalar.copy(
                        U[:, 2 * k : 2 * k + 2].rearrange(
                            "p a n c -> p (a n c)"),
                        pv[:].rearrange("p a b -> p (a b)"))
                    yield
            conv_tile.out = {}

            def emit_out(t, V):
                Vf = V[:].rearrange("p n c -> p (n c)")
                for h in range(2):
                    tp = ptr.tile([128, 128], f32, tag="tp", name=f"tp{t}_{h}")
                    nc.tensor.transpose(
                        tp[:], Vf[:, h * 128 : (h + 1) * 128], ident_sb[:]
                    )
                    nc.scalar.copy(ob[h][:, t * TP : (t + 1) * TP], tp[:])
                    nc.sync.dma_start(
                        out_d.ap()[h][:, t * TP : (t + 1) * TP],
                        ob[h][:, t * TP : (t + 1) * TP],
                    )

            def mul_accum_ci(t, it, U, R, S_ps):
                # tmp = U * R_bc; S_ps[p,no,co](psum) = sum_ci tmp; the DVE
                # product and the PE accumulation pipeline per ci-half.
                tmp = bigp.tile([128, CI, NO, CO], bf16, tag="tmp",
                                name=f"tmpm{t}_{it}")
                halves = (2,) if not CFG["fuse_halves"] else (0, 1)
                if not CFG["fuse_halves"]:
                    nc.vector.tensor_mul(
                        tmp[:], U[:],
                        R[:].unsqueeze(2).broadcast_to([128, CI, NO, CO]))
                    yield
                    for ci in range(CI):
                        nc.tensor.matmul(
                            S_ps[:], identb_sb[:], tmp[:, ci],
                            start=(ci == 0), stop=(ci == CI - 1))
                        if ci % 8 == 7:
                            yield
                    return
                for h in halves:
                    sl = slice(h * 16, (h + 1) * 16)
                    nc.vector.tensor_mul(
                        tmp[:, sl], U[:, sl],
                        R[:, sl].unsqueeze(2).broadcast_to([128, 16, NO, CO]),
                    )
                    yield
                    for ci in range(h * 16, (h + 1) * 16):
                        nc.tensor.matmul(
                            S_ps[:], identb_sb[:], tmp[:, ci],
                            start=(ci == 0), stop=(ci == CI - 1),
                        )
                    yield

            def mul_accum_no(t, it, U, V, L_ps, start, stop):
                # tmp = U * V_bc; L_ps[p,ci,co](psum) += sum_no tmp,
                # pipelined per ci-half.
                tmp = bigp.tile([128, CI, NO, CO], bf16, tag="tmp",
                                name=f"tmpd{t}_{it}")
                if not CFG["fuse_halves"]:
                    nc.vector.tensor_mul(
                        tmp[:], U[:],
                        V[:].unsqueeze(1).broadcast_to([128, CI, NO, CO]))
                    yield
                    for h in range(2):
                        sl = slice(h * 16, (h + 1) * 16)
                        Lh = L_ps[:, sl]
                        for no in range(NO):
                            nc.tensor.matmul(
                                Lh, identb_sb[:], tmp[:, sl, no],
                                start=(start and no == 0),
                                stop=(stop and no == NO - 1))
                        yield
                    return
                for h in range(2):
                    sl = slice(h * 16, (h + 1) * 16)
                    nc.vector.tensor_mul(
                        tmp[:, sl], U[:, sl],
                        V[:].unsqueeze(1).broadcast_to([128, 16, NO, CO]),
                    )
                    yield
                    Lh = L_ps[:, sl]
                    for no in range(NO):
                        nc.tensor.matmul(
                            Lh, identb_sb[:], tmp[:, sl, no],
                            start=(start and no == 0),
                            stop=(stop and no == NO - 1),
                        )
                    yield

            def squash(t, S, it, out_dtype):
                # S: [128, NO, CO] f32 preactivation ap (SBUF or PSUM)
                sq = statep.tile([128, NO, CO], f32, tag="sq", name=f"sq{t}_{it}")
                nc.scalar.activation(
                    sq[:].rearrange("p n c -> p (n c)"),
                    S.rearrange("p n c -> p (n c)"), AF.Square)
                nsq = statep.tile([128, CO], f32, tag="nsq", name=f"nsq{t}_{it}")
                nc.vector.tensor_reduce(
                    nsq[:], sq[:].transpose([0, 2, 1]), axis=AX.X, op=OP.add
                )
                yield
                lg = statep.tile([128, CO], f32, tag="lg", name=f"lg{t}_{it}")
                nc.scalar.activation(lg[:], nsq[:], AF.Ln, bias=eps_sb[:])
                sqr = statep.tile([128, CO], f32, tag="sqr", name=f"sqr{t}_{it}")
                nc.scalar.activation(sqr[:], lg[:], AF.Exp, scale=0.5)
                den = statep.tile([128, CO], f32, tag="den", name=f"den{t}_{it}")
                nc.scalar.activation(den[:], nsq[:], AF.Identity, bias=1.0)
                rcd = statep.tile([128, CO], f32, tag="rcd", name=f"rcd{t}_{it}")
                nc.vector.reciprocal(rcd[:], den[:])
                yield
                scl = statep.tile([128, CO], f32, tag="scl", name=f"scl{t}_{it}")
                nc.vector.tensor_mul(scl[:], sqr[:], rcd[:])
                V = statep.tile([128, NO, CO], out_dtype, tag=f"V{it}",
                                name=f"V{t}_{it}")
                eng = nc.gpsimd if CFG["vmul_pool"] else nc.vector
                eng.tensor_mul(
                    V[:], S, scl[:].unsqueeze(1).broadcast_to([128, NO, CO])
                )
                yield
                squash.out = V

            def routing_tile(t, U, Uxs):
                # ---- iteration 1: route is uniform 1/CI ----
                S1 = statep.tile([128, NO, CO], f32, tag="S", name=f"S1_{t}")
                if use_bias:
                    nc.vector.scalar_tensor_tensor(
                        S1[:].rearrange("p n c -> p (n c)"), Uxs[:], 1.0 / CI,
                        bias_sb[:], op0=OP.mult, op1=OP.add,
                    )
                else:
                    nc.scalar.activation(
                        S1[:].rearrange("p n c -> p (n c)"), Uxs[:],
                        AF.Copy, scale=1.0 / CI)
                yield
                yield from squash(t, S1[:], 1, bf16)
                V1 = squash.out
                # ---- logits L1 = sum_no U*V1 accumulated in PSUM ----
                L_ps = plog.tile([128, CI, CO], f32, tag="L", name=f"L{t}")
                yield from mul_accum_no(t, 1, U, V1, L_ps, start=True,
                                        stop=False)
                V = None
                for it in (2, 3):
                    # ---- softmax over co (logits read from PSUM) ----
                    E = statep.tile([128, CI, CO], bf16, tag="E",
                                    name=f"E{t}_{it}")
                    nc.scalar.activation(
                        E[:].rearrange("p a c -> p (a c)"),
                        L_ps[:].rearrange("p a c -> p (a c)"), AF.Exp)
                    sume = statep.tile([128, CI], f32, tag="sume",
                                       name=f"sume{t}_{it}")
                    nc.vector.tensor_reduce(sume[:], E[:], axis=AX.X, op=OP.add)
                    rec = statep.tile([128, CI], f32, tag="rec",
                                      name=f"rec{t}_{it}")
                    nc.vector.reciprocal(rec[:], sume[:])
                    recx = statep.tile([128, CI], bf16, tag="recb",
                                       name=f"recb{t}_{it}")
                    nc.scalar.copy(recx[:], rec[:])
                    yield
                    R = statep.tile([128, CI, CO], bf16, tag="R",
                                    name=f"R{t}_{it}")
                    nc.vector.tensor_mul(
                        R[:], E[:],
                        recx[:].unsqueeze(2).broadcast_to([128, CI, CO]),
                    )
                    yield
                    # ---- preactivation: sum_ci R * U  (PE accumulate) ----
                    S_ps = pacc.tile([128, NO, CO], f32, tag="Sp",
                                     name=f"Sp{t}_{it}")
                    yield from mul_accum_ci(t, it, U, R, S_ps)
                    if use_bias:
                        S = statep.tile([128, NO, CO], f32, tag="S",
                                        name=f"S{t}_{it}")
                        nc.vector.tensor_add(
                            S[:].rearrange("p n c -> p (n c)"),
                            S_ps[:].rearrange("p n c -> p (n c)"), bias_sb[:])
                        S_ap = S[:]
                    else:
                        S_ap = S_ps[:]
                    yield from squash(t, S_ap, it, bf16 if it == 2 else f32)
                    V = squash.out
                    if it == 2:
                        # ---- distances accumulate onto logits in PSUM ----
                        yield from mul_accum_no(t, 2, U, V, L_ps,
                                                start=False, stop=True)
                emit_out(t, V)

            def drain(gens):
                alive = [g for g in gens if g is not None]
                while alive:
                    for g in list(alive):
                        try:
                            next(g)
                        except StopIteration:
                            alive.remove(g)

            # Staggered software pipeline, one global drain. Invariant
            # kept from the working pair schedule: conv(t) is fully
            # emitted before routing(t) starts emitting. Routing starts
            # are offset by `stagger` rounds so in-flight tiles sit in
            # different phases; conv(t) is admitted late in routing(t-D)
            # so its evacuations never sit in the ACT queue ahead of
            # iter-3 ops they transitively depend on.
            def delayed(k, factory):
                def gen():
                    for _ in range(k):
                        yield
                    yield from factory()
                return gen()

            D = max(1, min(CFG["depth"], TILES))
            P = CFG["stagger"]
            CLAG = CFG["conv_lag"]
            CONVLEN = 18
            starts_c = {}
            starts_r = {}
            for t in range(TILES):
                starts_c[t] = t * 7 if t < D else starts_r[t - D] + CLAG
                prev = CONVLEN if t == 0 else starts_r[t - 1] + P
                starts_r[t] = max(starts_c[t] + CONVLEN, prev)
            gens = [
                delayed(starts_c[t],
                        (lambda tt: (lambda: conv_tile(tt)))(t))
                for t in range(TILES)
            ]
            gens += [
                delayed(starts_r[t],
                        (lambda tt: (lambda: routing_tile(
                            tt, *conv_tile.out[tt])))(t))
                for t in range(TILES)
            ]
            drain(gens)

    nc.compile()
    _BUILT[key] = nc
    return nc


def _assemble(out_halves_all):
    o = out_halves_all.reshape(-1, 2, 4, CO, NPIX)
    return np.ascontiguousarray(
        o.transpose(0, 3, 1, 2, 4).reshape(-1, CO, NO, H, W)
    )


def kernel(x, conv_w, bias):
    import sys
    if "/opt/trn_rl_repo" not in sys.path:
        sys.path.insert(0, "/opt/trn_rl_repo")
    from concourse import bass_utils

    patches, w_m, bias_bc, ident, ident_bf = _host_prep(x, conv_w, bias)
    nc = _build_nc(zero_bias=not np.any(bias_bc))
    in_maps = [
        {"patches": patches[b], "w": w_m, "bias": bias_bc, "ident": ident,
         "identb": ident_bf}
        for b in range(BS)
    ]
    res = bass_utils.run_bass_kernel_spmd(nc, in_maps, core_ids=list(range(BS)))
    outs = np.stack([r["out"] for r in res.results])
    return _assemble(outs).astype(np.float32)
